# revision 14
# baseline (speedup 1.0000x reference)
"""Trainium2 Bass kernel for single-head cross-attention (nn_Attention_16106127360635).

Computes, per batch b (8 batches, one per NeuronCore):
    w      = word_embs[b] @ W.T                    (2048, 1024)
    scores = w @ sentence_embs[b].T                (2048, 512)
    att    = softmax(scores, axis=-1)
    g      = att @ sentence_embs[b]                (2048, 1024)
    out    = concat([w, g], -1)                    (2048, 2048)

Strategy:
  - Data-parallel over batch: core b handles batch b.
  - fp16 matmul inputs (full PE rate, ~8x finer mantissa than bf16), fp32 PSUM
    accumulation, fp32 softmax. Measured end-to-end rel err ~2e-3.
  - scores computed via associativity: scores = word @ V with V = W.T @ sent.T,
    so `w` is only ever needed in its natural [s, f] output layout.
  - All inputs cast fp32->fp16 during contiguous SWDGE loads (no DRAM
    staging); every transpose (word^T, W^T, sent^T, att^T) is a 128x128 PE
    transpose feeding fp16 PSUM, drained by DVE copies. word^T is produced
    s-chunk-major so the first s-tiles' matmuls unlock after 1/4 of the
    transposes.
  - softmax: DVE row-max (negated) -> ACT exp with bias + accumulated row sum ->
    DVE reciprocal; normalization folded into the PSUM->SBUF copy of g as a
    per-partition scale.
  - g is not written separately: host slices it from comb[:, 1024:].
"""

import numpy as np

B = 8
S = 2048  # words (rows)
N = 512   # sentences
E = 1024  # embedding
P = 128

_NC = None


def _build():
    import concourse.mybir as mybir
    import concourse.tile as tile
    from concourse import bacc

    f32 = mybir.dt.float32
    f16 = mybir.dt.float16
    AX = mybir.AxisListType.X
    AF = mybir.ActivationFunctionType

    nc = bacc.Bacc("TRN2", target_bir_lowering=False, debug=False, num_devices=B)

    word = nc.dram_tensor("word", (S, E), f32, kind="ExternalInput").ap()
    sent = nc.dram_tensor("sent", (N, E), f32, kind="ExternalInput").ap()
    Win = nc.dram_tensor("w_in", (E, E), f32, kind="ExternalInput").ap()
    ident = nc.dram_tensor("ident", (P, P), f16, kind="ExternalInput").ap()
    comb = nc.dram_tensor("comb", (S, 2 * E), f32, kind="ExternalOutput").ap()

    EC = E // P   # 8 e-chunks
    FC = E // P   # 8 f-chunks
    NC_ = N // P  # 4 n-chunks
    NST = S // P  # 16 s-tiles

    with tile.TileContext(nc) as tc:
        with (
            tc.tile_pool(name="big", bufs=1) as bigp,
            tc.tile_pool(name="wn", bufs=1) as wnp,
            tc.tile_pool(name="work", bufs=2) as wp,
            tc.tile_pool(name="stat", bufs=4) as statp,
            tc.tile_pool(name="ps", bufs=2, space="PSUM") as psp,
        ):
            identsb = bigp.tile([P, P], f16, name="identsb", tag="ident")
            nc.sync.dma_start(out=identsb[:], in_=ident)

            # ---- contiguous SWDGE cast-loads (fp32 DRAM -> fp16 SBUF) ----
            # Each load grabs several 128-row chunks via a [128, k, E] AP so
            # the Q7 descriptor generator (~1us per dma_start) isn't pacing
            # the pipeline. sent arrives first (gates the first transposes).
            # A row-chunk group [r*P:(r+k)*P] lands as tile[:, i*E:(i+1)*E].
            def cast_load(tile_ap, src, r0, k):
                view = src.rearrange("(c p) e -> p c e", p=P)[:, r0 : r0 + k, :]
                nc.gpsimd.dma_start(out=tile_ap, in_=view)

            wordnat4 = [None] * 4
            wn_a = wnp.tile([P, 2 * E], f16, name="wn_a", tag="wn_a")
            cast_load(wn_a[:], word, 0, 2)
            wn_b = wnp.tile([P, 2 * E], f16, name="wn_b", tag="wn_b")
            cast_load(wn_b[:], word, 2, 2)
            sentnat2 = []
            for q in range(2):
                t_ = bigp.tile([P, 2 * E], f16, name=f"sentnat2_{q}", tag=f"sentnat2_{q}")
                cast_load(t_[:], sent, 2 * q, 2)
                sentnat2.append(t_)
            sentnat = [sentnat2[j // 2][:, (j % 2) * E : (j % 2 + 1) * E]
                       for j in range(NC_)]
            Wnat2 = []
            for r in range(2):
                t2 = bigp.tile([P, 4 * E], f16, name=f"Wnat2_{r}", tag=f"Wnat2_{r}")
                cast_load(t2[:], Win, 4 * r, 4)
                Wnat2.append(t2)
            for q in range(1, 4):
                wordnat4[q] = wnp.tile([P, 4 * E], f16, name=f"wordnat4_{q}",
                                       tag=f"wn{q}")
                cast_load(wordnat4[q][:], word, 4 * q, 4)
            wordnat = (
                [wn_a[:, 0:E], wn_a[:, E:2 * E], wn_b[:, 0:E], wn_b[:, E:2 * E]]
                + [wordnat4[t // 4][:, (t % 4) * E : (t % 4 + 1) * E]
                   for t in range(4, NST)]
            )
            Wnat = [Wnat2[j // 4][:, (j % 4) * E : (j % 4 + 1) * E]
                    for j in range(FC)]

            # transpose-drain helper: alternate DVE/ACT, rotate psum across
            # the (pre-loop idle) 'w' bank pair and the 'at' pair
            _tr_n = [0]
            def drain_copy(dst_ap, src_ap):
                if _tr_n[0] % 2 == 0:
                    nc.vector.tensor_copy(dst_ap, src_ap)
                else:
                    nc.scalar.copy(dst_ap, src_ap)
                _tr_n[0] += 1
            def tr_psum(name):
                tag = "w" if _tr_n[0] % 2 == 0 else "at"
                return psp.tile([P, N], f16, name=name, tag=tag, bufs=2)

            # ---- word^T first quarter: earliest possible PE work ----
            wordT = []
            for j in range(EC):
                t_ = bigp.tile([P, S], f16, name=f"wordT{j}", tag=f"wordT{j}")
                wordT.append(t_)
            for half in range(2):
                for ec in range(EC):
                    pt = tr_psum(f"pwT0_{half}_{ec}")
                    for ti in range(2):
                        nc.tensor.transpose(
                            pt[:, ti * P : (ti + 1) * P],
                            wordnat[half * 2 + ti][:, ec * P : (ec + 1) * P],
                            identsb[:],
                        )
                    drain_copy(
                        wordT[ec][:, half * 2 * P : (half * 2 + 2) * P],
                        pt[:, 0 : 2 * P],
                    )

            # ---- PE transposes of the small operands ----
            # sent^T chunks [e=128, n=512] (moving operand of the V matmul)
            sentT = []
            for j in range(EC):
                pt = tr_psum(f"psentT{j}")
                for i in range(NC_):
                    nc.tensor.transpose(
                        pt[:, i * P : (i + 1) * P],
                        sentnat[i][:, j * P : (j + 1) * P],
                        identsb[:],
                    )
                t_ = bigp.tile([P, N], f16, name=f"sentT{j}", tag=f"sentT{j}")
                drain_copy(t_[:], pt[:])
                sentT.append(t_)
            # W^T chunks [e=128, f=1024] (moving operand of the w matmul)
            WT = []
            for j in range(EC):
                t_ = bigp.tile([P, E], f16, name=f"WT{j}", tag=f"WT{j}")
                for q in range(2):
                    pt = tr_psum(f"pWT{j}_{q}")
                    for i in range(4):
                        nc.tensor.transpose(
                            pt[:, i * P : (i + 1) * P],
                            Wnat[q * 4 + i][:, j * P : (j + 1) * P],
                            identsb[:],
                        )
                    drain_copy(t_[:, q * N : (q + 1) * N], pt[:])
                WT.append(t_)

            # ---- V = W^T @ sent^T : V[e, n], chunks [128, 512] ----
            Vt = []
            for et in range(EC):
                pv = psp.tile([P, N], f32, name=f"pv{et}", tag="g", bufs=2)
                for fc in range(FC):
                    nc.tensor.matmul(
                        pv[:],
                        lhsT=Wnat[fc][:, et * P : (et + 1) * P],
                        rhs=sentT[fc][:],
                        start=(fc == 0),
                        stop=(fc == FC - 1),
                    )
                v = bigp.tile([P, N], f16, name=f"V{et}", tag=f"V{et}")
                nc.vector.tensor_copy(v[:], pv[:])
                Vt.append(v)

            # ---- word^T quarters tq1..3 (tq0 hoisted above) ----
            for tq in range(1, 4):
                for ec in range(EC):
                    pt = tr_psum(f"pwT{tq}_{ec}")
                    for ti in range(4):
                        nc.tensor.transpose(
                            pt[:, ti * P : (ti + 1) * P],
                            wordnat[tq * 4 + ti][:, ec * P : (ec + 1) * P],
                            identsb[:],
                        )
                    drain_copy(wordT[ec][:, tq * N : (tq + 1) * N], pt[:])

            # ---- main loop over 16 s-tiles ----
            for s in range(NST):
                ssl = slice(s * P, (s + 1) * P)
                # w halves and scores share the stationary word^T chunk
                pw0 = psp.tile([P, 512], f32, name=f"pw{s}_0", tag="w", bufs=2)
                pw1 = psp.tile([P, 512], f32, name=f"pw{s}_1", tag="w", bufs=2)
                psc = psp.tile([P, N], f32, name=f"psc{s}", tag="sc", bufs=2)
                for ec in range(EC):
                    st, sp = (ec == 0), (ec == EC - 1)
                    lhsT = wordT[ec][:, ssl]
                    nc.tensor.matmul(psc[:], lhsT=lhsT, rhs=Vt[ec][:],
                                     start=st, stop=sp)
                    nc.tensor.matmul(pw0[:], lhsT=lhsT, rhs=WT[ec][:, 0:512],
                                     start=st, stop=sp)
                    nc.tensor.matmul(pw1[:], lhsT=lhsT, rhs=WT[ec][:, 512:1024],
                                     start=st, stop=sp)
                for h, pw in ((0, pw0), (1, pw1)):
                    wsb = wp.tile([P, 512], f32, name=f"wsb{s}_{h}", tag="wsb", bufs=3)
                    nc.scalar.copy(wsb[:], pw[:])
                    nc.sync.dma_start(
                        out=comb[ssl, h * 512 : (h + 1) * 512], in_=wsb[:]
                    )

                # softmax pieces
                negmax = statp.tile([P, 1], f32, name=f"negmax{s}", tag="negmax")
                nc.vector.reduce_max(negmax[:], psc[:], axis=AX, negate=True)
                att = wp.tile([P, N], f16, name=f"att{s}", tag="att", bufs=2)
                sumexp = statp.tile([P, 1], f32, name=f"sumexp{s}", tag="sumexp")
                nc.scalar.activation(
                    att[:], psc[:], AF.Exp, bias=negmax[:], scale=1.0,
                    accum_out=sumexp[:],
                )
                recip = statp.tile([P, 1], f32, name=f"recip{s}", tag="recip")
                nc.vector.reciprocal(recip[:], sumexp[:])

                # att^T via PE transpose (psum tile dtype must match input: fp16)
                pat = psp.tile([P, N], f16, name=f"pat{s}", tag="at", bufs=2)
                for j in range(NC_):
                    nc.tensor.transpose(
                        pat[:, j * P : (j + 1) * P],
                        att[:, j * P : (j + 1) * P],
                        identsb[:],
                    )
                attT = wp.tile([P, N], f16, name=f"attT{s}", tag="attT", bufs=2)
                nc.vector.tensor_copy(attT[:], pat[:])

                # g = att @ sent -> psum [s=128, e=512] x2; normalize on copy-out
                for h in range(2):
                    pg = psp.tile([P, 512], f32, name=f"pg{s}_{h}", tag="g", bufs=2)
                    for j in range(NC_):
                        nc.tensor.matmul(
                            pg[:],
                            lhsT=attT[:, j * P : (j + 1) * P],
                            rhs=sentnat[j][:, h * 512 : (h + 1) * 512],
                            start=(j == 0),
                            stop=(j == NC_ - 1),
                        )
                    gsb = wp.tile([P, 512], f32, name=f"gsb{s}_{h}", tag="gsb", bufs=3)
                    nc.scalar.activation(gsb[:], pg[:], AF.Copy, scale=recip[:])
                    nc.sync.dma_start(
                        out=comb[ssl, E + h * 512 : E + (h + 1) * 512], in_=gsb[:]
                    )

    nc.compile()
    return nc


def run(word_embs, sentence_embs, W, trace=False):
    """Build (cached), run on 8 cores, return (comb[8,2048,2048], results)."""
    global _NC
    from concourse import bass_utils

    if _NC is None:
        _NC = _build()

    ident = np.eye(P, dtype=np.float16)
    Wc = np.ascontiguousarray(W, dtype=np.float32)
    in_maps = [
        {
            "word": np.ascontiguousarray(word_embs[b], dtype=np.float32),
            "sent": np.ascontiguousarray(sentence_embs[b], dtype=np.float32),
            "w_in": Wc,
            "ident": ident,
        }
        for b in range(B)
    ]
    results = bass_utils.run_bass_kernel_spmd(
        _NC, in_maps, core_ids=list(range(B)), trace=trace
    )
    comb = np.stack([results.results[b]["comb"] for b in range(B)])
    return comb, results


def kernel(word_embs, sentence_embs, W):
    comb, _ = run(word_embs, sentence_embs, W)
    g = np.ascontiguousarray(comb[:, :, E:])
    return comb, g


# revision 24
# speedup vs baseline: 1.7175x; 1.7175x over previous
"""Trainium2 Bass kernel for single-head cross-attention (nn_Attention_16106127360635).

Computes, per batch b (8 batches, one per NeuronCore):
    w      = word_embs[b] @ W.T                    (2048, 1024)
    scores = w @ sentence_embs[b].T                (2048, 512)
    att    = softmax(scores, axis=-1)
    g      = att @ sentence_embs[b]                (2048, 1024)
    out    = concat([w, g], -1)                    (2048, 2048)

Strategy:
  - Data-parallel over batch: core b handles batch b.
  - fp16 matmul inputs (full PE rate, ~8x finer mantissa than bf16), fp32 PSUM
    accumulation, fp32 softmax. Measured end-to-end rel err ~2e-3.
  - scores computed via associativity: scores = word @ V with V = W.T @ sent.T,
    so `w` is only ever needed in its natural [s, f] output layout.
  - All inputs cast fp32->fp16 during contiguous SWDGE loads (no DRAM
    staging); every transpose (word^T, W^T, sent^T, att^T) is a 128x128 PE
    transpose feeding fp16 PSUM, drained by DVE copies. word^T is produced
    s-chunk-major so the first s-tiles' matmuls unlock after 1/4 of the
    transposes.
  - softmax: DVE row-max (negated) -> ACT exp with bias + accumulated row sum ->
    DVE reciprocal; normalization folded into the PSUM->SBUF copy of g as a
    per-partition scale.
  - g is not written separately: host slices it from comb[:, 1024:].
"""

import numpy as np

B = 8
S = 2048  # words (rows)
N = 512   # sentences
E = 1024  # embedding
P = 128

_NC = None


def _build():
    import concourse.mybir as mybir
    import concourse.tile as tile
    from concourse import bacc

    f32 = mybir.dt.float32
    f16 = mybir.dt.float16
    AX = mybir.AxisListType.X
    AF = mybir.ActivationFunctionType

    nc = bacc.Bacc("TRN2", target_bir_lowering=False, debug=False, num_devices=B)

    word = nc.dram_tensor("word", (S, E), f32, kind="ExternalInput").ap()
    sent = nc.dram_tensor("sent", (N, E), f32, kind="ExternalInput").ap()
    Win = nc.dram_tensor("w_in", (E, E), f32, kind="ExternalInput").ap()
    ident = nc.dram_tensor("ident", (P, P), f16, kind="ExternalInput").ap()
    comb = nc.dram_tensor("comb", (S, 2 * E), f32, kind="ExternalOutput").ap()

    EC = E // P   # 8 e-chunks
    FC = E // P   # 8 f-chunks
    NC_ = N // P  # 4 n-chunks
    NST = S // P  # 16 s-tiles

    with tile.TileContext(nc) as tc:
        with (
            tc.tile_pool(name="dram", bufs=1, space="DRAM") as dpool,
            tc.tile_pool(name="big", bufs=1) as bigp,
            tc.tile_pool(name="wn", bufs=1) as wnp,
            tc.tile_pool(name="work", bufs=2) as wp,
            tc.tile_pool(name="stat", bufs=4) as statp,
            tc.tile_pool(name="ps", bufs=2, space="PSUM") as psp,
        ):
            identsb = bigp.tile([P, P], f16, name="identsb", tag="ident")
            nc.sync.dma_start(out=identsb[:], in_=ident)

            # PE warm-up: ~3us of dep-free matmuls while the first loads are
            # in flight, so the HAM clock gate is at full rate (and the cost
            # model's ramp is past) when real work starts.
            pwu = psp.tile([P, P], f32, name="pwu", tag="sc", bufs=2)
            for _ in range(30):
                nc.tensor.matmul(pwu[:], lhsT=identsb[:], rhs=identsb[:],
                                 start=True, stop=True)

            # ---- contiguous SWDGE cast-loads (fp32 DRAM -> fp16 SBUF) ----
            # Each load grabs several 128-row chunks via a [128, k, E] AP so
            # the Q7 descriptor generator (~1us per dma_start) isn't pacing
            # the pipeline. A row-chunk group [r*P:(r+k)*P] lands as
            # tile[:, i*E:(i+1)*E].
            def cast_load(tile_ap, src, r0, k):
                view = src.rearrange("(c p) e -> p c e", p=P)[:, r0 : r0 + k, :]
                nc.gpsimd.dma_start(out=tile_ap, in_=view)

            wn_a = wnp.tile([P, E], f16, name="wn_a", tag="wn_a")
            cast_load(wn_a[:], word, 0, 1)
            wn_b = wnp.tile([P, 3 * E], f16, name="wn_b", tag="wn_b")
            cast_load(wn_b[:], word, 1, 3)
            wordnat = [wn_a[:, 0:E], wn_b[:, 0:E],
                       wn_b[:, E:2 * E], wn_b[:, 2 * E:3 * E]]

            sentnat2 = []
            for q in range(2):
                t_ = bigp.tile([P, 2 * E], f16, name=f"sentnat2_{q}", tag=f"sentnat2_{q}")
                cast_load(t_[:], sent, 2 * q, 2)
                sentnat2.append(t_)
            sentnat = [sentnat2[j // 2][:, (j % 2) * E : (j % 2 + 1) * E]
                       for j in range(NC_)]
            Wnat2 = []
            for r in range(2):
                t2 = bigp.tile([P, 4 * E], f16, name=f"Wnat2_{r}", tag=f"Wnat2_{r}")
                cast_load(t2[:], Win, 4 * r, 4)
                Wnat2.append(t2)
            Wnat = [Wnat2[j // 4][:, (j % 4) * E : (j % 4 + 1) * E]
                    for j in range(FC)]

            # transpose-drain helper: alternate DVE/ACT, rotate psum across
            # the (pre-loop idle) 'w' bank pair and the 'at' pair
            _tr_n = [0]
            def drain_copy(dst_ap, src_ap):
                if _tr_n[0] % 2 == 0:
                    nc.vector.tensor_copy(dst_ap, src_ap)
                else:
                    nc.scalar.copy(dst_ap, src_ap)
                _tr_n[0] += 1
            def tr_psum(name):
                tag = "w" if _tr_n[0] % 2 == 0 else "at"
                return psp.tile([P, N], f16, name=name, tag=tag, bufs=2)

            # ---- word^T hybrid ----
            # Rows 0..511 via PE transposes of natural cast-loads (available
            # within ~3us -> the first s-tiles' matmuls unlock early); rows
            # 512..2047 via DRAM fp16 staging + big DMA xbar transpose-loads
            # (frees ~10us of PE at the cost of DMA latency that the early
            # tiles' matmuls cover).
            wordT = []
            for j in range(EC):
                t_ = bigp.tile([P, S], f16, name=f"wordT{j}", tag=f"wordT{j}")
                wordT.append(t_)
            for ec in range(EC):
                pt = tr_psum(f"pwT0a_{ec}")
                nc.tensor.transpose(pt[:, 0:P], wordnat[0][:, ec * P : (ec + 1) * P],
                                    identsb[:])
                drain_copy(wordT[ec][:, 0:P], pt[:, 0:P])
            for ec in range(EC):
                pt = tr_psum(f"pwT0b_{ec}")
                for ti in range(3):
                    nc.tensor.transpose(
                        pt[:, ti * P : (ti + 1) * P],
                        wordnat[1 + ti][:, ec * P : (ec + 1) * P],
                        identsb[:],
                    )
                drain_copy(wordT[ec][:, P : 4 * P], pt[:, 0 : 3 * P])
            wordf16 = dpool.tile([S - 4 * P, E], f16, name="wordf16", tag="wordf16")
            nc.gpsimd.dma_start(out=wordf16[:], in_=word[4 * P :, :])
            for j in range(EC):
                nc.sync.dma_start(
                    out=wordT[j][:, 4 * P :],
                    in_=wordf16[:, j * P : (j + 1) * P],
                    transpose=True,
                )

            # ---- PE transposes of the small operands ----
            # sent^T chunks [e=128, n=512] (moving operand of the V matmul)
            sentT = []
            for j in range(EC):
                pt = tr_psum(f"psentT{j}")
                for i in range(NC_):
                    nc.tensor.transpose(
                        pt[:, i * P : (i + 1) * P],
                        sentnat[i][:, j * P : (j + 1) * P],
                        identsb[:],
                    )
                t_ = bigp.tile([P, N], f16, name=f"sentT{j}", tag=f"sentT{j}")
                drain_copy(t_[:], pt[:])
                sentT.append(t_)
            # W^T chunks [e=128, f=1024] (moving operand of the w matmul)
            WT = []
            for j in range(EC):
                t_ = bigp.tile([P, E], f16, name=f"WT{j}", tag=f"WT{j}")
                for q in range(2):
                    pt = tr_psum(f"pWT{j}_{q}")
                    for i in range(4):
                        nc.tensor.transpose(
                            pt[:, i * P : (i + 1) * P],
                            Wnat[q * 4 + i][:, j * P : (j + 1) * P],
                            identsb[:],
                        )
                    drain_copy(t_[:, q * N : (q + 1) * N], pt[:])
                WT.append(t_)

            # ---- V = W^T @ sent^T : V[e, n], chunks [128, 512] ----
            Vt = []
            for et in range(EC):
                pv = psp.tile([P, N], f32, name=f"pv{et}", tag="g", bufs=2)
                for fc in range(FC):
                    nc.tensor.matmul(
                        pv[:],
                        lhsT=Wnat[fc][:, et * P : (et + 1) * P],
                        rhs=sentT[fc][:],
                        start=(fc == 0),
                        stop=(fc == FC - 1),
                    )
                v = bigp.tile([P, N], f16, name=f"V{et}", tag=f"V{et}")
                nc.vector.tensor_copy(v[:], pv[:])
                Vt.append(v)


            # ---- main loop over 16 s-tiles ----
            for s in range(NST):
                ssl = slice(s * P, (s + 1) * P)
                # w halves and scores share the stationary word^T chunk
                if s < NST - 2:
                    pw0 = psp.tile([P, 512], f32, name=f"pw{s}_0", tag="w", bufs=2)
                    pw1 = psp.tile([P, 512], f32, name=f"pw{s}_1", tag="w", bufs=2)
                psc = psp.tile([P, N], f32, name=f"psc{s}", tag="sc", bufs=2)
                defer_w = s >= NST - 2
                for ec in range(EC):
                    st, sp = (ec == 0), (ec == EC - 1)
                    lhsT = wordT[ec][:, ssl]
                    nc.tensor.matmul(psc[:], lhsT=lhsT, rhs=Vt[ec][:],
                                     start=st, stop=sp)
                    if not defer_w:
                        nc.tensor.matmul(pw0[:], lhsT=lhsT, rhs=WT[ec][:, 0:512],
                                         start=st, stop=sp)
                        nc.tensor.matmul(pw1[:], lhsT=lhsT, rhs=WT[ec][:, 512:1024],
                                         start=st, stop=sp)
                if not defer_w:
                    for h, pw in ((0, pw0), (1, pw1)):
                        wsb = wp.tile([P, 512], f32, name=f"wsb{s}_{h}",
                                      tag="wsb", bufs=4)
                        nc.vector.tensor_copy(wsb[:], pw[:])
                        nc.sync.dma_start(
                            out=comb[ssl, h * 512 : (h + 1) * 512], in_=wsb[:]
                        )

                # softmax pieces
                negmax = statp.tile([P, 1], f32, name=f"negmax{s}", tag="negmax")
                nc.vector.reduce_max(negmax[:], psc[:], axis=AX, negate=True)
                att = wp.tile([P, N], f16, name=f"att{s}", tag="att", bufs=3)
                sumexp = statp.tile([P, 1], f32, name=f"sumexp{s}", tag="sumexp")
                nc.scalar.activation(
                    att[:], psc[:], AF.Exp, bias=negmax[:], scale=1.0,
                    accum_out=sumexp[:],
                )
                recip = statp.tile([P, 1], f32, name=f"recip{s}", tag="recip")
                nc.vector.reciprocal(recip[:], sumexp[:])

                # att^T via PE transpose (psum tile dtype must match input: fp16)
                pat = psp.tile([P, N], f16, name=f"pat{s}", tag="at", bufs=2)
                for j in range(NC_):
                    nc.tensor.transpose(
                        pat[:, j * P : (j + 1) * P],
                        att[:, j * P : (j + 1) * P],
                        identsb[:],
                    )
                attT = wp.tile([P, N], f16, name=f"attT{s}", tag="attT", bufs=3)
                nc.vector.tensor_copy(attT[:], pat[:])

                # g = att @ sent -> psum [s=128, e=512] x2; normalize on copy-out
                for h in range(2):
                    pg = psp.tile([P, 512], f32, name=f"pg{s}_{h}", tag="g", bufs=2)
                    for j in range(NC_):
                        nc.tensor.matmul(
                            pg[:],
                            lhsT=attT[:, j * P : (j + 1) * P],
                            rhs=sentnat[j][:, h * 512 : (h + 1) * 512],
                            start=(j == 0),
                            stop=(j == NC_ - 1),
                        )
                    gsb = wp.tile([P, 512], f32, name=f"gsb{s}_{h}", tag="gsb", bufs=3)
                    nc.scalar.activation(gsb[:], pg[:], AF.Copy, scale=recip[:])
                    nc.sync.dma_start(
                        out=comb[ssl, E + h * 512 : E + (h + 1) * 512], in_=gsb[:]
                    )

            # deferred w matmuls for the last two s-tiles: they are the only
            # PE work independent of the final softmax chains, so putting
            # them last keeps the PE busy while those chains drain
            for s in range(NST - 2, NST):
                ssl = slice(s * P, (s + 1) * P)
                pw0 = psp.tile([P, 512], f32, name=f"pwd{s}_0", tag="w", bufs=2)
                pw1 = psp.tile([P, 512], f32, name=f"pwd{s}_1", tag="w", bufs=2)
                for h, pw in ((0, pw0), (1, pw1)):
                    for ec in range(EC):
                        nc.tensor.matmul(
                            pw[:], lhsT=wordT[ec][:, ssl],
                            rhs=WT[ec][:, h * 512 : (h + 1) * 512],
                            start=(ec == 0), stop=(ec == EC - 1),
                        )
                    wsb = wp.tile([P, 512], f32, name=f"wsbd{s}_{h}",
                                  tag="wsb", bufs=4)
                    nc.vector.tensor_copy(wsb[:], pw[:])
                    nc.sync.dma_start(
                        out=comb[ssl, h * 512 : (h + 1) * 512], in_=wsb[:]
                    )

    nc.compile()
    return nc


def run(word_embs, sentence_embs, W, trace=False):
    """Build (cached), run on 8 cores, return (comb[8,2048,2048], results)."""
    global _NC
    from concourse import bass_utils

    if _NC is None:
        _NC = _build()

    ident = np.eye(P, dtype=np.float16)
    Wc = np.ascontiguousarray(W, dtype=np.float32)
    in_maps = [
        {
            "word": np.ascontiguousarray(word_embs[b], dtype=np.float32),
            "sent": np.ascontiguousarray(sentence_embs[b], dtype=np.float32),
            "w_in": Wc,
            "ident": ident,
        }
        for b in range(B)
    ]
    results = bass_utils.run_bass_kernel_spmd(
        _NC, in_maps, core_ids=list(range(B)), trace=trace
    )
    comb = np.stack([results.results[b]["comb"] for b in range(B)])
    return comb, results


def kernel(word_embs, sentence_embs, W):
    comb, _ = run(word_embs, sentence_embs, W)
    g = np.ascontiguousarray(comb[:, :, E:])
    return comb, g


# revision 26
# speedup vs baseline: 51448.2974x; 29954.6314x over previous
"""Trainium2 Bass kernel for single-head cross-attention (nn_Attention_16106127360635).

Computes, per batch b (8 batches, one per NeuronCore):
    w      = word_embs[b] @ W.T                    (2048, 1024)
    scores = w @ sentence_embs[b].T                (2048, 512)
    att    = softmax(scores, axis=-1)
    g      = att @ sentence_embs[b]                (2048, 1024)
    out    = concat([w, g], -1)                    (2048, 2048)

Strategy:
  - Data-parallel over batch: core b handles batch b.
  - fp16 matmul inputs (full PE rate, ~8x finer mantissa than bf16), fp32 PSUM
    accumulation, fp32 softmax. Measured end-to-end rel err ~2e-3.
  - scores computed via associativity: scores = word @ V with V = W.T @ sent.T,
    so `w` is only ever needed in its natural [s, f] output layout.
  - Inputs cast fp32->fp16 during contiguous SWDGE loads. W^T, sent^T,
    att^T and the first quarter of word^T are built with 128x128 PE
    transposes (fp16 PSUM, drained by DVE/ACT copies); the remaining word^T
    rows go through a DRAM fp16 staging copy + big DMA xbar transpose-loads,
    trading DMA latency (covered by the early tiles' matmuls) for ~10us of
    PE time.
  - softmax: DVE row-max (negated) -> ACT exp with bias + accumulated row sum ->
    DVE reciprocal; normalization folded into the PSUM->SBUF copy of g as a
    per-partition scale.
  - g is not written separately: host slices it from comb[:, 1024:].
"""

import numpy as np

B = 8
S = 2048  # words (rows)
N = 512   # sentences
E = 1024  # embedding
P = 128

_NC = None


def _build():
    import concourse.mybir as mybir
    import concourse.tile as tile
    from concourse import bacc

    f32 = mybir.dt.float32
    f16 = mybir.dt.float16
    AX = mybir.AxisListType.X
    AF = mybir.ActivationFunctionType

    nc = bacc.Bacc("TRN2", target_bir_lowering=False, debug=False, num_devices=B)

    word = nc.dram_tensor("word", (S, E), f32, kind="ExternalInput").ap()
    sent = nc.dram_tensor("sent", (N, E), f32, kind="ExternalInput").ap()
    Win = nc.dram_tensor("w_in", (E, E), f32, kind="ExternalInput").ap()
    ident = nc.dram_tensor("ident", (P, P), f16, kind="ExternalInput").ap()
    comb = nc.dram_tensor("comb", (S, 2 * E), f32, kind="ExternalOutput").ap()

    EC = E // P   # 8 e-chunks
    FC = E // P   # 8 f-chunks
    NC_ = N // P  # 4 n-chunks
    NST = S // P  # 16 s-tiles

    with tile.TileContext(nc) as tc:
        with (
            tc.tile_pool(name="dram", bufs=1, space="DRAM") as dpool,
            tc.tile_pool(name="big", bufs=1) as bigp,
            tc.tile_pool(name="wn", bufs=1) as wnp,
            tc.tile_pool(name="work", bufs=2) as wp,
            tc.tile_pool(name="stat", bufs=4) as statp,
            tc.tile_pool(name="ps", bufs=2, space="PSUM") as psp,
        ):
            identsb = bigp.tile([P, P], f16, name="identsb", tag="ident")
            nc.sync.dma_start(out=identsb[:], in_=ident)

            # PE warm-up: ~3us of dep-free matmuls while the first loads are
            # in flight, so the HAM clock gate is at full rate (and the cost
            # model's ramp is past) when real work starts.
            pwu = psp.tile([P, P], f32, name="pwu", tag="sc", bufs=2)
            for _ in range(30):
                nc.tensor.matmul(pwu[:], lhsT=identsb[:], rhs=identsb[:],
                                 start=True, stop=True)

            # ---- contiguous SWDGE cast-loads (fp32 DRAM -> fp16 SBUF) ----
            # Each load grabs several 128-row chunks via a [128, k, E] AP so
            # the Q7 descriptor generator (~1us per dma_start) isn't pacing
            # the pipeline. A row-chunk group [r*P:(r+k)*P] lands as
            # tile[:, i*E:(i+1)*E].
            def cast_load(tile_ap, src, r0, k):
                view = src.rearrange("(c p) e -> p c e", p=P)[:, r0 : r0 + k, :]
                nc.gpsimd.dma_start(out=tile_ap, in_=view)

            wn_a = wnp.tile([P, E], f16, name="wn_a", tag="wn_a")
            cast_load(wn_a[:], word, 0, 1)
            wn_b = wnp.tile([P, 3 * E], f16, name="wn_b", tag="wn_b")
            cast_load(wn_b[:], word, 1, 3)
            wordnat = [wn_a[:, 0:E], wn_b[:, 0:E],
                       wn_b[:, E:2 * E], wn_b[:, 2 * E:3 * E]]

            sentnat2 = []
            for q in range(2):
                t_ = bigp.tile([P, 2 * E], f16, name=f"sentnat2_{q}", tag=f"sentnat2_{q}")
                cast_load(t_[:], sent, 2 * q, 2)
                sentnat2.append(t_)
            sentnat = [sentnat2[j // 2][:, (j % 2) * E : (j % 2 + 1) * E]
                       for j in range(NC_)]
            Wnat2 = []
            for r in range(2):
                t2 = bigp.tile([P, 4 * E], f16, name=f"Wnat2_{r}", tag=f"Wnat2_{r}")
                cast_load(t2[:], Win, 4 * r, 4)
                Wnat2.append(t2)
            Wnat = [Wnat2[j // 4][:, (j % 4) * E : (j % 4 + 1) * E]
                    for j in range(FC)]

            # transpose-drain helper: alternate DVE/ACT, rotate psum across
            # the (pre-loop idle) 'w' bank pair and the 'at' pair
            _tr_n = [0]
            def drain_copy(dst_ap, src_ap):
                if _tr_n[0] % 2 == 0:
                    nc.vector.tensor_copy(dst_ap, src_ap)
                else:
                    nc.scalar.copy(dst_ap, src_ap)
                _tr_n[0] += 1
            def tr_psum(name):
                tag = "w" if _tr_n[0] % 2 == 0 else "at"
                return psp.tile([P, N], f16, name=name, tag=tag, bufs=2)

            # ---- word^T hybrid ----
            # Rows 0..511 via PE transposes of natural cast-loads (available
            # within ~3us -> the first s-tiles' matmuls unlock early); rows
            # 512..2047 via DRAM fp16 staging + big DMA xbar transpose-loads
            # (frees ~10us of PE at the cost of DMA latency that the early
            # tiles' matmuls cover).
            wordT = []
            for j in range(EC):
                t_ = bigp.tile([P, S], f16, name=f"wordT{j}", tag=f"wordT{j}")
                wordT.append(t_)
            for ec in range(EC):
                pt = tr_psum(f"pwT0a_{ec}")
                nc.tensor.transpose(pt[:, 0:P], wordnat[0][:, ec * P : (ec + 1) * P],
                                    identsb[:])
                drain_copy(wordT[ec][:, 0:P], pt[:, 0:P])
            for ec in range(EC):
                pt = tr_psum(f"pwT0b_{ec}")
                for ti in range(3):
                    nc.tensor.transpose(
                        pt[:, ti * P : (ti + 1) * P],
                        wordnat[1 + ti][:, ec * P : (ec + 1) * P],
                        identsb[:],
                    )
                drain_copy(wordT[ec][:, P : 4 * P], pt[:, 0 : 3 * P])
            wordf16 = dpool.tile([S - 4 * P, E], f16, name="wordf16", tag="wordf16")
            nc.gpsimd.dma_start(out=wordf16[:], in_=word[4 * P :, :])
            for j in range(EC):
                nc.sync.dma_start(
                    out=wordT[j][:, 4 * P :],
                    in_=wordf16[:, j * P : (j + 1) * P],
                    transpose=True,
                )

            # ---- PE transposes of the small operands ----
            # sent^T chunks [e=128, n=512] (moving operand of the V matmul)
            sentT = []
            for j in range(EC):
                pt = tr_psum(f"psentT{j}")
                for i in range(NC_):
                    nc.tensor.transpose(
                        pt[:, i * P : (i + 1) * P],
                        sentnat[i][:, j * P : (j + 1) * P],
                        identsb[:],
                    )
                t_ = bigp.tile([P, N], f16, name=f"sentT{j}", tag=f"sentT{j}")
                drain_copy(t_[:], pt[:])
                sentT.append(t_)
            # W^T chunks [e=128, f=1024] (moving operand of the w matmul)
            WT = []
            for j in range(EC):
                t_ = bigp.tile([P, E], f16, name=f"WT{j}", tag=f"WT{j}")
                for q in range(2):
                    pt = tr_psum(f"pWT{j}_{q}")
                    for i in range(4):
                        nc.tensor.transpose(
                            pt[:, i * P : (i + 1) * P],
                            Wnat[q * 4 + i][:, j * P : (j + 1) * P],
                            identsb[:],
                        )
                    drain_copy(t_[:, q * N : (q + 1) * N], pt[:])
                WT.append(t_)

            # ---- V = W^T @ sent^T : V[e, n], chunks [128, 512] ----
            Vt = []
            for et in range(EC):
                pv = psp.tile([P, N], f32, name=f"pv{et}", tag="g", bufs=2)
                for fc in range(FC):
                    nc.tensor.matmul(
                        pv[:],
                        lhsT=Wnat[fc][:, et * P : (et + 1) * P],
                        rhs=sentT[fc][:],
                        start=(fc == 0),
                        stop=(fc == FC - 1),
                    )
                v = bigp.tile([P, N], f16, name=f"V{et}", tag=f"V{et}")
                nc.vector.tensor_copy(v[:], pv[:])
                Vt.append(v)


            # ---- main loop over 16 s-tiles ----
            for s in range(NST):
                ssl = slice(s * P, (s + 1) * P)
                # w halves and scores share the stationary word^T chunk
                if s < NST - 2:
                    pw0 = psp.tile([P, 512], f32, name=f"pw{s}_0", tag="w", bufs=2)
                    pw1 = psp.tile([P, 512], f32, name=f"pw{s}_1", tag="w", bufs=2)
                psc = psp.tile([P, N], f32, name=f"psc{s}", tag="sc", bufs=2)
                defer_w = s >= NST - 2
                for ec in range(EC):
                    st, sp = (ec == 0), (ec == EC - 1)
                    lhsT = wordT[ec][:, ssl]
                    nc.tensor.matmul(psc[:], lhsT=lhsT, rhs=Vt[ec][:],
                                     start=st, stop=sp)
                    if not defer_w:
                        nc.tensor.matmul(pw0[:], lhsT=lhsT, rhs=WT[ec][:, 0:512],
                                         start=st, stop=sp)
                        nc.tensor.matmul(pw1[:], lhsT=lhsT, rhs=WT[ec][:, 512:1024],
                                         start=st, stop=sp)
                if not defer_w:
                    for h, pw in ((0, pw0), (1, pw1)):
                        wsb = wp.tile([P, 512], f32, name=f"wsb{s}_{h}",
                                      tag="wsb", bufs=4)
                        nc.vector.tensor_copy(wsb[:], pw[:])
                        nc.sync.dma_start(
                            out=comb[ssl, h * 512 : (h + 1) * 512], in_=wsb[:]
                        )

                # softmax pieces
                negmax = statp.tile([P, 1], f32, name=f"negmax{s}", tag="negmax")
                nc.vector.reduce_max(negmax[:], psc[:], axis=AX, negate=True)
                att = wp.tile([P, N], f16, name=f"att{s}", tag="att", bufs=3)
                sumexp = statp.tile([P, 1], f32, name=f"sumexp{s}", tag="sumexp")
                nc.scalar.activation(
                    att[:], psc[:], AF.Exp, bias=negmax[:], scale=1.0,
                    accum_out=sumexp[:],
                )
                recip = statp.tile([P, 1], f32, name=f"recip{s}", tag="recip")
                nc.vector.reciprocal(recip[:], sumexp[:])

                # att^T via PE transpose (psum tile dtype must match input: fp16)
                pat = psp.tile([P, N], f16, name=f"pat{s}", tag="at", bufs=2)
                for j in range(NC_):
                    nc.tensor.transpose(
                        pat[:, j * P : (j + 1) * P],
                        att[:, j * P : (j + 1) * P],
                        identsb[:],
                    )
                attT = wp.tile([P, N], f16, name=f"attT{s}", tag="attT", bufs=3)
                nc.vector.tensor_copy(attT[:], pat[:])

                # g = att @ sent -> psum [s=128, e=512] x2; normalize on copy-out
                for h in range(2):
                    pg = psp.tile([P, 512], f32, name=f"pg{s}_{h}", tag="g", bufs=2)
                    for j in range(NC_):
                        nc.tensor.matmul(
                            pg[:],
                            lhsT=attT[:, j * P : (j + 1) * P],
                            rhs=sentnat[j][:, h * 512 : (h + 1) * 512],
                            start=(j == 0),
                            stop=(j == NC_ - 1),
                        )
                    gsb = wp.tile([P, 512], f32, name=f"gsb{s}_{h}", tag="gsb", bufs=3)
                    nc.scalar.activation(gsb[:], pg[:], AF.Copy, scale=recip[:])
                    nc.sync.dma_start(
                        out=comb[ssl, E + h * 512 : E + (h + 1) * 512], in_=gsb[:]
                    )

            # deferred w matmuls for the last two s-tiles: they are the only
            # PE work independent of the final softmax chains, so putting
            # them last keeps the PE busy while those chains drain
            for s in range(NST - 2, NST):
                ssl = slice(s * P, (s + 1) * P)
                pw0 = psp.tile([P, 512], f32, name=f"pwd{s}_0", tag="w", bufs=2)
                pw1 = psp.tile([P, 512], f32, name=f"pwd{s}_1", tag="w", bufs=2)
                for h, pw in ((0, pw0), (1, pw1)):
                    for ec in range(EC):
                        nc.tensor.matmul(
                            pw[:], lhsT=wordT[ec][:, ssl],
                            rhs=WT[ec][:, h * 512 : (h + 1) * 512],
                            start=(ec == 0), stop=(ec == EC - 1),
                        )
                    wsb = wp.tile([P, 512], f32, name=f"wsbd{s}_{h}",
                                  tag="wsb", bufs=4)
                    nc.vector.tensor_copy(wsb[:], pw[:])
                    nc.sync.dma_start(
                        out=comb[ssl, h * 512 : (h + 1) * 512], in_=wsb[:]
                    )

    nc.compile()
    return nc


def run(word_embs, sentence_embs, W, trace=False):
    """Build (cached), run on 8 cores, return (comb[8,2048,2048], results)."""
    global _NC
    from concourse import bass_utils

    if _NC is None:
        _NC = _build()

    ident = np.eye(P, dtype=np.float16)
    Wc = np.ascontiguousarray(W, dtype=np.float32)
    in_maps = [
        {
            "word": np.ascontiguousarray(word_embs[b], dtype=np.float32),
            "sent": np.ascontiguousarray(sentence_embs[b], dtype=np.float32),
            "w_in": Wc,
            "ident": ident,
        }
        for b in range(B)
    ]
    results = bass_utils.run_bass_kernel_spmd(
        _NC, in_maps, core_ids=list(range(B)), trace=trace
    )
    comb = np.stack([results.results[b]["comb"] for b in range(B)])
    return comb, results


def kernel(word_embs, sentence_embs, W):
    comb, _ = run(word_embs, sentence_embs, W)
    g = np.ascontiguousarray(comb[:, :, E:])
    return comb, g
